# revision 1
# baseline (speedup 1.0000x reference)
"""Original staged baseline + bf16 output DMA (halves the output-DMA tail)."""

import math
from contextlib import ExitStack

import numpy as np
import ml_dtypes

import concourse.bass as bass
import concourse.mybir as mybir
import concourse.tile as tile
from concourse import bacc
from concourse.bass_utils import run_bass_kernel_spmd
P = 128
HEADS = 16
DH = 64
D = 1024          # model dim
INNER = 1024      # heads * dh
SCALE = DH ** -0.5
N_CORES = 8
RPB = 4           # ranks (cores) per batch
BF16 = mybir.dt.bfloat16
F32 = mybir.dt.float32


def _build(mq: int, qe: int | None = None, dbg: bool = False):
    """Build the per-core SPMD graph for mq queries/core (mq % 32 == 0)."""
    qe = mq if qe is None else qe   # active query columns (<= mq)
    Kk = RPB * mq               # key slots per batch, multiple of 128
    VW = DH + 1                 # V~ cols per head
    nkt = Kk // P               # 128-row key tiles
    TT = math.ceil(qe / P)      # query-token tiles per core
    KCH = 384                   # K^T free-dim chunk (psum-bank friendly)
    nkch = math.ceil(Kk / KCH)

    def tspan(tt):
        return min(P, qe - tt * P)

    nc = bacc.Bacc(None, target_bir_lowering=False, num_devices=N_CORES)

    xt_in = nc.declare_dram_parameter("xt", [D, Kk], BF16, isOutput=False)
    wqkv_in = nc.declare_dram_parameter("wqkv", [D, 3 * INNER], BF16, isOutput=False)
    wout_in = nc.declare_dram_parameter("wout", [INNER, D], BF16, isOutput=False)
    npad_in = nc.declare_dram_parameter("npad", [1, 1], F32, isOutput=False)
    out_ext = nc.declare_dram_parameter("out", [qe, D], BF16, isOutput=True)

    with tile.TileContext(nc) as tc, ExitStack() as ctx:
        sb = ctx.enter_context(tc.tile_pool(name="sb", bufs=1))
        ps = ctx.enter_context(tc.tile_pool(name="ps", bufs=1, space="PSUM"))

        npad_sb = sb.tile([1, 1], F32, tag="npad", bufs=1, name="npad_sb")
        nc.sync.dma_start(npad_sb[:], npad_in[:])

        # HAM warm-up: dependency-free matmuls on zeros so the PE clock is at
        # 2.4 GHz when the first real (DMA-gated) matmuls issue.
        warm = sb.tile([P, 512], BF16, tag="warm", bufs=1, name="warm")
        nc.vector.memset(warm[:], 0.0)
        wps = ps.tile([DH + 1, 512], F32, tag="av", bufs=2, name="wps")
        for i in range(20):
            nc.tensor.matmul(wps[:], warm[:, 0:DH + 1], warm[:],
                             start=True, stop=True)


        # ---- inputs: x^T (full batch + own query slice), weights.
        # Round-robin the DMA issues over four sequencers (issue costs ~600ns
        # serially per sequencer); wk+xt first, they gate the first matmul.
        seqs = [nc.sync, nc.scalar, nc.gpsimd]
        _n = [0]

        def dma(dst, src):
            seqs[_n[0] % len(seqs)].dma_start(dst, src)
            _n[0] += 1

        xt, wk, wv, wq = [], [], [], []
        for kc in range(8):
            tq_ = sb.tile([P, INNER], BF16, tag="wq", bufs=8, name=f"wq{kc}")
            dma(tq_[:, 0:512], wqkv_in[kc * P:(kc + 1) * P, 0:512])
            dma(tq_[:, 512:1024], wqkv_in[kc * P:(kc + 1) * P, 512:INNER])
            wq.append(tq_)
            # the core's own query chunk sits in cols 0:qe of its xt (host
            # orders each core's keys own-chunk-first), so Q needs only this
            # small prefix - load it first
            t_ = sb.tile([P, Kk], BF16, tag="xt", bufs=8, name=f"xt{kc}")
            dma(t_[:, 0:qe], xt_in[kc * P:(kc + 1) * P, 0:qe])
            xt.append(t_)
        for kc in range(8):
            tk = sb.tile([P, INNER], BF16, tag="wk", bufs=8, name=f"wk{kc}")
            dma(tk[:, 0:512], wqkv_in[kc * P:(kc + 1) * P, INNER:INNER + 512])
            dma(tk[:, 512:1024], wqkv_in[kc * P:(kc + 1) * P, INNER + 512:2 * INNER])
            wk.append(tk)
            dma(xt[kc][:, qe:Kk], xt_in[kc * P:(kc + 1) * P, qe:Kk])
        for kc in range(8):
            tv = sb.tile([P, INNER], BF16, tag="wv", bufs=8, name=f"wv{kc}")
            dma(tv[:], wqkv_in[kc * P:(kc + 1) * P, 2 * INNER:3 * INNER])
            wv.append(tv)

        wout_sb = []
        for t in range(8):
            tw = sb.tile([P, D], BF16, tag="wout", bufs=8, name=f"wo{t}")
            dma(tw[:], wout_in[t * P:(t + 1) * P, :])
            wout_sb.append(tw)

        # ---- attention.  CH=2 staging groups = exactly one key-tile (both
        # heads), 3 staging slots, AV deferred one group so the PE never waits
        # on the exp.  Pair 0 is interleaved into the V phase above via
        # pair_step (its AV(kt) only needs vt[kt]); pairs 1-7 run after.
        CH = 2
        aoT = []
        pair_state = {}

        pending = [None]   # one AV group deferred globally (across pairs)

        def pair_begin(hp):
            avp_a = ps.tile([P, qe], F32, tag="av", bufs=2, name=f"av{hp}a")
            avp_b = ps.tile([P, qe], F32, tag="av", bufs=2, name=f"av{hp}b")
            pair_state[hp] = {"avp": [avp_a, avp_b]}

        def emit_av(hp, grp, pt_):
            avp = pair_state[hp]["avp"]
            for j, (kt, h) in enumerate(grp):
                nc.tensor.matmul(
                    avp[h][0:VW, :],
                    vt[kt][:, (2 * hp + h) * VW:(2 * hp + h) * VW + VW],
                    pt_[:, j * qe:(j + 1) * qe],
                    start=(kt == 0), stop=(kt == nkt - 1),
                    skip_group_check=True)

        def pair_step(hp, kt, defer=False):
            st = pair_state[hp]
            grp = [(kt, 0), (kt, 1)]
            sps = ps.tile([P, CH * 512], F32, tag="ss", bufs=3, name=f"sps{hp}_{kt}")
            for j, (kt_, h) in enumerate(grp):
                nc.tensor.matmul(sps[:, j * 512: j * 512 + qe],
                                 kf[hp][:, kt_ * P:(kt_ + 1) * P],
                                 qtz[2 * hp + h][:],
                                 start=True, stop=True)
            pt_ = sb.tile([P, CH * qe], BF16, tag="pt", bufs=6, name=f"pt{hp}_{kt}")
            nc.scalar.activation(
                pt_[:].rearrange("p (u c) -> p u c", c=qe)[:, 0:len(grp), :],
                sps[:].rearrange("p (u c) -> p u c", c=512)[:, 0:len(grp), 0:qe],
                mybir.ActivationFunctionType.Exp, scale=SCALE)
            if pending[0] is not None:
                emit_av(*pending[0])
            pending[0] = (hp, grp, pt_)

        def flush_av():
            if pending[0] is not None:
                emit_av(*pending[0])
                pending[0] = None

        def pair_end(hp):
            st = pair_state[hp]
            avp_a, avp_b = st["avp"]
            # softmax denominators ride in row 64; subtract the pad count,
            # reciprocal, broadcast across the 64 head dims, scale, pack.
            ao = sb.tile([P, qe], BF16, tag="aoT", bufs=8, name=f"aoT{hp}")
            for h, avp_h in ((1, avp_b), (0, avp_a)):
                av_ = sb.tile([DH + 1, qe], F32, tag="aos", bufs=6, name=f"aos{hp}_{h}")
                nc.vector.tensor_copy(av_[:], avp_h[0:DH + 1, :])
                den = sb.tile([1, qe], F32, tag="den", bufs=6, name=f"den{hp}_{h}")
                nc.vector.tensor_scalar(den[:], av_[DH:DH + 1, :], npad_sb[0:1, 0:1],
                                        None, op0=mybir.AluOpType.subtract)
                rec = sb.tile([1, qe], F32, tag="rec", bufs=6, name=f"rec{hp}_{h}")
                nc.vector.reciprocal_approx_fast(rec[:], den[:])
                fac = sb.tile([DH, qe], F32, tag="fac", bufs=6, name=f"fac{hp}_{h}")
                nc.gpsimd.partition_broadcast(fac[:], rec[:])
                if h == 0:
                    nc.vector.tensor_tensor(ao[0:DH, :], av_[0:DH, :], fac[:],
                                            op=mybir.AluOpType.mult)
                else:
                    tmpb = sb.tile([DH, qe], BF16, tag="tmpb", bufs=4,
                                   name=f"tmpb{hp}")
                    nc.vector.tensor_tensor(tmpb[:], av_[0:DH, :], fac[:],
                                            op=mybir.AluOpType.mult)
                    # partition shift 0:64 -> 64:128 needs a DMA, not DVE
                    nc.sync.dma_start(ao[DH:P, :], tmpb[:])
            aoT.append(ao)


        # ---- Q^T (own slice), zero-padded per head: qtz[h] has head h's 64
        # dims in their packed partition rows, zeros in the other 64, so S^T
        # contracts over the full 128 rows sharing one K^T lhsT per head pair.
        qtz = [None] * HEADS
        for t in range(8):
            qps = ps.tile([P, qe], F32, tag="ss", bufs=3, name=f"qps{t}")
            kcs = [(t + i) % 8 for i in range(8)]
            for i, kc in enumerate(kcs):
                nc.tensor.matmul(qps[:], wq[kc][:, t * P:(t + 1) * P], xt[kc][:, 0:qe],
                                 start=(i == 0), stop=(i == 7))
            a = sb.tile([P, qe], BF16, tag="qtz", bufs=HEADS, name=f"qtz{2 * t}")
            nc.vector.memset(a[64:128, :], 0.0)
            nc.vector.tensor_copy(a[0:64, :], qps[0:64, :])
            qtz[2 * t] = a
            b = sb.tile([P, qe], BF16, tag="qtz", bufs=HEADS, name=f"qtz{2 * t + 1}")
            nc.vector.memset(b[0:64, :], 0.0)
            nc.vector.tensor_copy(b[64:128, :], qps[64:128, :])
            qtz[2 * t + 1] = b

        # ---- K^T for the whole batch: kf[t] [128 featdims, Kk keys] bf16
        defer_pairs = []
        kf = []
        for t in range(8):
            kft = sb.tile([P, Kk], BF16, tag="kf", bufs=8, name=f"kf{t}")
            for ch in range(nkch):
                w_ = min(KCH, Kk - ch * KCH)
                kps = ps.tile([P, KCH], F32, tag="ss", bufs=3, name=f"kps{t}_{ch}")
                kcs = [(t + ch + i) % 8 for i in range(8)]
                for i, kc in enumerate(kcs):
                    nc.tensor.matmul(kps[:, 0:w_], wk[kc][:, t * P:(t + 1) * P],
                                     xt[kc][:, ch * KCH: ch * KCH + w_],
                                     start=(i == 0), stop=(i == 7))
                nc.vector.tensor_copy(kft[:, ch * KCH: ch * KCH + w_], kps[:, 0:w_])
            kf.append(kft)

        # ---- V~ for the whole batch: vt[kt] [128 keys, 16*(64+1)] bf16 with a
        # ones column per head (softmax denominator rides row 64 of AV psum).
        vt = []
        pair0_started = False
        for kt in range(nkt):
            t_ = sb.tile([P, HEADS * VW], BF16, tag="vt", bufs=nkt, name=f"vt{kt}")
            nc.gpsimd.memset(t_[:, :], 0.0)
            nc.gpsimd.memset(
                t_[:].rearrange("p (h c) -> p h c", c=VW)[:, :, DH:DH + 1], 1.0)
            for nf in range(2):
                vps = ps.tile([P, 512], F32, tag="ss", bufs=3, name=f"vps{kt}_{nf}")
                kcs = [(kt + nf + i) % 8 for i in range(8)]
                for i, kc in enumerate(kcs):
                    nc.tensor.matmul(vps[:], xt[kc][:, kt * P:(kt + 1) * P],
                                     wv[kc][:, nf * 512:(nf + 1) * 512],
                                     start=(i == 0), stop=(i == 7))
                nc.vector.tensor_copy(
                    t_[:].rearrange("p (h c) -> p h c", c=VW)[:, nf * 8:(nf + 1) * 8, 0:DH],
                    vps[:].rearrange("p (h c) -> p h c", c=DH))
            vt.append(t_)
            if not pair0_started:
                pair_begin(0)
                pair0_started = True
            pair_step(0, kt)

        for hp in range(1, 8):
            pair_begin(hp)
            for kt in range(nkt):
                pair_step(hp, kt)
                if kt == 1:
                    pair_end(hp - 1)
        flush_av()
        pair_end(7)

        # ---- out projection.
        for mt in range(TT):
            pm = tspan(mt)
            osb = sb.tile([P, D], BF16, tag="osb", bufs=3, name=f"osb{mt}")
            for nf in range(2):
                op_ps = ps.tile([P, 512], F32, tag="ss", bufs=3, name=f"op{mt}_{nf}")
                ts_ = list(range(8))
                for i, t in enumerate(ts_):
                    nc.tensor.matmul(op_ps[0:pm, :], aoT[t][:, mt * P: mt * P + pm],
                                     wout_sb[t][:, nf * 512:(nf + 1) * 512],
                                     start=(i == 0), stop=(i == 7))
                nc.vector.tensor_copy(osb[0:pm, nf * 512:(nf + 1) * 512],
                                      op_ps[0:pm, :])
                for h in range(2):
                    sl = slice(nf * 512 + h * 256, nf * 512 + (h + 1) * 256)
                    dma(out_ext[mt * P: mt * P + pm, sl], osb[0:pm, sl])

    nc.compile()
    return nc


_GRAPH_CACHE: dict = {}


def _get_graph(mq: int, qe: int):
    if (mq, qe) not in _GRAPH_CACHE:
        _GRAPH_CACHE[(mq, qe)] = _build(mq, qe)
    return _GRAPH_CACHE[(mq, qe)]


def kernel(x, mask, W_qkv, W_out):
    x = np.asarray(x, dtype=np.float32)
    mask = np.asarray(mask, dtype=np.float32)
    W_qkv = np.asarray(W_qkv, dtype=np.float32)
    W_out = np.asarray(W_out, dtype=np.float32)
    b, n, d = x.shape
    assert (b, d) == (2, D) and W_qkv.shape == (D, 3 * INNER)

    idx = [np.nonzero(mask[i] > 0.5)[0] for i in range(b)]
    m = [len(ix) for ix in idx]
    mq = max(32, math.ceil(max(m) / RPB / 32) * 32)
    Kk = RPB * mq
    # real tokens spread evenly over the 4 cores of each batch group so every
    # core computes at most qe active query columns
    chunks = [np.array_split(ix, RPB) for ix in idx]
    qe = max(4, math.ceil(max(len(c) for cs in chunks for c in cs) / 4) * 4)

    nc = _get_graph(mq, qe)

    bf16 = ml_dtypes.bfloat16
    xg = np.zeros((b, RPB, mq, d), dtype=np.float32)
    for i in range(b):
        for r in range(RPB):
            xg[i, r, :len(chunks[i][r])] = x[i][chunks[i][r]]
    xg8 = xg.astype(bf16)
    wqkv_bf = W_qkv.astype(bf16)
    wout_bf = W_out.astype(bf16)

    in_maps = []
    for core in range(N_CORES):
        bi, r = divmod(core, RPB)
        # key order is permutation-invariant; put this core's own query chunk
        # first so the device reads queries from xt[:, 0:qe] at a fixed offset
        order = [r] + [j for j in range(RPB) if j != r]
        xtc = np.ascontiguousarray(
            xg8[bi][order].reshape(Kk, d).transpose(1, 0))  # [D, Kk]
        in_maps.append({
            "xt": xtc,
            "wqkv": wqkv_bf,
            "wout": wout_bf,
            "npad": np.array([[Kk - m[bi]]], dtype=np.float32),
        })

    res = run_bass_kernel_spmd(nc, in_maps, core_ids=list(range(N_CORES)))

    out = np.zeros((b, n, d), dtype=np.float32)
    for bi in range(b):
        for r in range(RPB):
            ch = chunks[bi][r]
            out[bi][ch] = np.asarray(res.results[bi * RPB + r]["out"][:len(ch)], dtype=np.float32)
    return out



# revision 4
# speedup vs baseline: 1.1997x; 1.1997x over previous
"""Head-sharded tensor-parallel attention (2 heads/core, 8 cores).

Each core computes QKV for its 2 heads over ALL active tokens of both
batches, full attention for those heads, and a partial output
``AO_c @ W_out[c's 128 inner dims, :]``.  The host sums the 8 partial
outputs (the unshard step of the W_out-row sharding) and scatters back
into masked positions.  Masked-out rows of the reference output are
exactly zero, so only active tokens are processed (gathered on host);
pad keys have x=0 -> k=0 -> exp(0)=1, corrected by subtracting the pad
count from the softmax denominator.
"""

import math
from contextlib import ExitStack

import numpy as np
import ml_dtypes

import concourse.bass as bass
import concourse.mybir as mybir
import concourse.tile as tile
from concourse import bacc
from concourse.bass_utils import run_bass_kernel_spmd

P = 128
D = 1024          # model dim
HEADS = 16
DH = 64
VW = DH + 1       # v columns per head + ones column (softmax denominator)
SCALE = DH ** -0.5
N_CORES = 8
BF16 = mybir.dt.bfloat16
F32 = mybir.dt.float32


def _chunks(total, step):
    out = []
    o = 0
    while o < total:
        out.append((o, min(step, total - o)))
        o += step
    return out


def _build(T: int):
    """Per-core SPMD graph; T = padded token count per batch (mult of 128)."""
    nkt = T // P
    NT = 2 * T
    QCS = _chunks(T, 512)      # query chunks within a batch
    nmt = NT // P              # output token tiles

    nc = bacc.Bacc(None, target_bir_lowering=False, num_devices=N_CORES)

    xt_in = nc.declare_dram_parameter("xt", [D, NT], BF16, isOutput=False)
    wqkv_in = nc.declare_dram_parameter("wqkv", [D, 384], BF16, isOutput=False)
    wout_in = nc.declare_dram_parameter("wout", [P, D], BF16, isOutput=False)
    npad_in = nc.declare_dram_parameter("npad", [1, 2], F32, isOutput=False)
    out_ext = nc.declare_dram_parameter("out", [NT, D], BF16, isOutput=True)

    with tile.TileContext(nc) as tc, ExitStack() as ctx:
        sb = ctx.enter_context(tc.tile_pool(name="sb", bufs=1))
        ps = ctx.enter_context(tc.tile_pool(name="ps", bufs=1, space="PSUM"))

        npad_sb = sb.tile([1, 2], F32, tag="npad", bufs=1, name="npad_sb")
        nc.sync.dma_start(npad_sb[:], npad_in[:])

        # HAM warm-up: dependency-free matmuls so the PE clock ramps while
        # the first DMAs land.
        warm = sb.tile([P, 512], BF16, tag="warm", bufs=1, name="warm")
        nc.vector.memset(warm[:], 0.0)
        for i in range(18):
            wps = ps.tile([P, 512], F32, tag="ss", bufs=2, name=f"wps{i}")
            nc.tensor.matmul(wps[:], warm[:, 0:P], warm[:],
                             start=True, stop=True, skip_group_check=True)

        # ---- input DMAs, round-robined over sequencers.
        seqs = [nc.sync, nc.scalar, nc.gpsimd]
        _n = [0]

        def dma(dst, src, seq=None):
            (seqs[_n[0] % len(seqs)] if seq is None else seq).dma_start(dst, src)
            _n[0] += 1

        wqkv_sb = []
        for kc in range(8):
            tw = sb.tile([P, 384], BF16, tag="wqkv", bufs=8, name=f"wqkv{kc}")
            dma(tw[:], wqkv_in[kc * P:(kc + 1) * P, :])
            wqkv_sb.append(tw)
        xt = []
        for kc in range(8):
            t_ = sb.tile([P, NT], BF16, tag="xt", bufs=8, name=f"xt{kc}")
            dma(t_[:, 0:T], xt_in[kc * P:(kc + 1) * P, 0:T])
            xt.append(t_)
        for kc in range(8):
            dma(xt[kc][:, T:NT], xt_in[kc * P:(kc + 1) * P, T:NT])
        wout_sb = sb.tile([P, D], BF16, tag="wout", bufs=1, name="wout_sb")
        dma(wout_sb[:, 0:512], wout_in[:, 0:512])
        dma(wout_sb[:, 512:D], wout_in[:, 512:D])

        # ---- K^T and Q^T per batch: [128 dims(2 heads), T] bf16 in SBUF.
        kf = [None, None]
        qt = [None, None]

        def proj_kq(b):
            kfb = sb.tile([P, T], BF16, tag="kf", bufs=2, name=f"kf{b}")
            qtb = sb.tile([P, T], BF16, tag="qt", bufs=2, name=f"qt{b}")
            for dst, col0 in ((kfb, 128), (qtb, 0)):
                for qo, qw in QCS:
                    pps = ps.tile([P, 512], F32, tag="ss", bufs=2,
                                  name=f"pp{b}_{col0}_{qo}")
                    for i in range(8):
                        kc = (i + qo // 512) % 8
                        nc.tensor.matmul(
                            pps[:, 0:qw],
                            wqkv_sb[kc][:, col0:col0 + P],
                            xt[kc][:, b * T + qo: b * T + qo + qw],
                            start=(i == 0), stop=(i == 7))
                    nc.vector.tensor_copy(dst[:, qo:qo + qw], pps[:, 0:qw])
            kf[b] = kfb
            qt[b] = qtb

        # ---- V tiles per (batch, kt): [128 keys, 2*VW] bf16 with ones col.
        vt = [[None] * nkt, [None] * nkt]

        def proj_v(b):
            for kt in range(nkt):
                t_ = sb.tile([P, 2 * VW], BF16, tag="vt", bufs=2 * nkt,
                             name=f"vt{b}_{kt}")
                nc.gpsimd.memset(
                    t_[:].rearrange("p (h c) -> p h c", c=VW)[:, :, DH:DH + 1], 1.0)
                vps = ps.tile([P, P], F32, tag="ss", bufs=2, name=f"vps{b}_{kt}")
                for i in range(8):
                    kc = (i + kt) % 8
                    nc.tensor.matmul(
                        vps[:],
                        xt[kc][:, b * T + kt * P: b * T + (kt + 1) * P],
                        wqkv_sb[kc][:, 256:384],
                        start=(i == 0), stop=(i == 7))
                nc.vector.tensor_copy(
                    t_[:].rearrange("p (h c) -> p h c", c=VW)[:, :, 0:DH],
                    vps[:].rearrange("p (h c) -> p h c", c=DH))
                vt[b][kt] = t_

        # ---- S + exp stream for one batch.  S^T tiles [keys, queries] per
        # (kt, qchunk); both heads share one 2-bank psum tile; exp writes a
        # per-kt SBUF tile pt[b][kt] = [128, 2*T] bf16 (head-major halves).
        pt = [[None] * nkt, [None] * nkt]

        def s_exp(b):
            for kt in range(nkt):
                ptt = sb.tile([P, 2 * T], BF16, tag="pt", bufs=nkt + 5,
                              name=f"pt{b}_{kt}")
                pt[b][kt] = ptt
                for qo, qw in QCS:
                    sps = ps.tile([P, 1024], F32, tag="sps", bufs=2,
                                  name=f"sps{b}_{kt}_{qo}")
                    for h in range(2):
                        nc.tensor.matmul(
                            sps[:, h * 512: h * 512 + qw],
                            kf[b][h * DH:(h + 1) * DH, kt * P:(kt + 1) * P],
                            qt[b][h * DH:(h + 1) * DH, qo:qo + qw],
                            start=True, stop=True, skip_group_check=True)
                    nc.scalar.activation(
                        ptt[:].rearrange("p (u c) -> p u c", c=T)[:, 0:2, qo:qo + qw],
                        sps[:].rearrange("p (u c) -> p u c", c=512)[:, 0:2, 0:qw],
                        mybir.ActivationFunctionType.Exp, scale=SCALE)

        # ---- AV pass + normalize for one batch -> aoT[b] [128, T] bf16.
        aoT = [None, None]

        def av_norm(b):
            aob = sb.tile([P, T], BF16, tag="aoT", bufs=2, name=f"aoT{b}")
            aoT[b] = aob
            for h in range(2):
                av_ = sb.tile([VW, T], F32, tag="avs", bufs=2, name=f"avs{b}_{h}")
                for qo, qw in QCS:
                    avp = ps.tile([P, 512], F32, tag="av", bufs=2,
                                  name=f"avp{b}_{h}_{qo}")
                    for kt in range(nkt):
                        nc.tensor.matmul(
                            avp[0:VW, 0:qw],
                            vt[b][kt][:, h * VW:(h + 1) * VW],
                            pt[b][kt][:, h * T + qo: h * T + qo + qw],
                            start=(kt == 0), stop=(kt == nkt - 1),
                            skip_group_check=True)
                    nc.vector.tensor_copy(av_[:, qo:qo + qw], avp[0:VW, 0:qw])
                den = sb.tile([1, T], F32, tag="den", bufs=4, name=f"den{b}_{h}")
                nc.vector.tensor_scalar(den[:], av_[DH:DH + 1, :],
                                        npad_sb[0:1, b:b + 1], None,
                                        op0=mybir.AluOpType.subtract)
                rec = sb.tile([1, T], F32, tag="rec", bufs=4, name=f"rec{b}_{h}")
                nc.vector.reciprocal_approx_fast(rec[:], den[:])
                fac = sb.tile([DH, T], F32, tag="fac", bufs=2, name=f"fac{b}_{h}")
                nc.gpsimd.partition_broadcast(fac[:], rec[:])
                if h == 0:
                    nc.vector.tensor_tensor(aob[0:DH, :], av_[0:DH, :], fac[:],
                                            op=mybir.AluOpType.mult)
                else:
                    tmpb = sb.tile([DH, T], BF16, tag="tmpb", bufs=2,
                                   name=f"tmpb{b}")
                    nc.vector.tensor_tensor(tmpb[:], av_[0:DH, :], fac[:],
                                            op=mybir.AluOpType.mult)
                    # partition shift 0:64 -> 64:128 needs a DMA, not DVE
                    nc.sync.dma_start(aob[DH:P, :], tmpb[:])

        # ---- partial out-projection for one batch: [T, 1024] bf16 -> DRAM.
        def out_proj(b):
            for mt in range(nkt):
                osb = sb.tile([P, D], BF16, tag="osb", bufs=3, name=f"osb{b}_{mt}")
                for nf in range(2):
                    ops = ps.tile([P, 512], F32, tag="ss", bufs=2,
                                  name=f"op{b}_{mt}_{nf}")
                    nc.tensor.matmul(ops[:], aob_slice(b, mt),
                                     wout_sb[:, nf * 512:(nf + 1) * 512],
                                     start=True, stop=True, skip_group_check=True)
                    if nf == 0:
                        nc.vector.tensor_copy(osb[:, 0:512], ops[:])
                    else:
                        nc.scalar.activation(osb[:, 512:D], ops[:],
                                             mybir.ActivationFunctionType.Copy)
                dma(out_ext[b * T + mt * P: b * T + (mt + 1) * P, 0:512],
                    osb[:, 0:512], seq=nc.sync)
                dma(out_ext[b * T + mt * P: b * T + (mt + 1) * P, 512:D],
                    osb[:, 512:D], seq=nc.gpsimd)

        def aob_slice(b, mt):
            return aoT[b][:, mt * P:(mt + 1) * P]

        # ---- schedule (program order = scheduler priority)
        proj_kq(0)
        s_exp(0)       # streams on ACT while PE continues below
        proj_v(0)
        proj_kq(1)
        proj_v(1)
        av_norm(0)
        s_exp(1)
        out_proj(0)
        av_norm(1)
        out_proj(1)

    nc.compile()
    return nc


_GRAPH_CACHE: dict = {}


def _get_graph(T: int):
    if T not in _GRAPH_CACHE:
        _GRAPH_CACHE[T] = _build(T)
    return _GRAPH_CACHE[T]


def kernel(x, mask, W_qkv, W_out):
    x = np.asarray(x, dtype=np.float32)
    mask = np.asarray(mask, dtype=np.float32)
    W_qkv = np.asarray(W_qkv, dtype=np.float32)
    W_out = np.asarray(W_out, dtype=np.float32)
    b, n, d = x.shape
    assert (b, d) == (2, D) and W_qkv.shape == (D, 3 * D)

    idx = [np.nonzero(mask[i] > 0.5)[0] for i in range(b)]
    m = [len(ix) for ix in idx]
    nkt = max(1, math.ceil(max(m) / P))
    T = nkt * P

    nc = _get_graph(T)

    bf16 = ml_dtypes.bfloat16
    xg = np.zeros((b, T, d), dtype=np.float32)
    for i in range(b):
        xg[i, :m[i]] = x[i][idx[i]]
    xt_all = np.ascontiguousarray(
        xg.reshape(b * T, d).transpose(1, 0)).astype(bf16)   # [D, 2T]
    npad = np.array([[T - m[0], T - m[1]]], dtype=np.float32)

    in_maps = []
    for c in range(N_CORES):
        cols = slice(c * P, (c + 1) * P)
        wqkv_c = np.ascontiguousarray(np.concatenate(
            [W_qkv[:, 0 * D:1 * D][:, cols],
             W_qkv[:, 1 * D:2 * D][:, cols],
             W_qkv[:, 2 * D:3 * D][:, cols]], axis=1)).astype(bf16)
        wout_c = np.ascontiguousarray(W_out[cols, :]).astype(bf16)
        in_maps.append({
            "xt": xt_all,
            "wqkv": wqkv_c,
            "wout": wout_c,
            "npad": npad,
        })

    res = run_bass_kernel_spmd(nc, in_maps, core_ids=list(range(N_CORES)))

    total = np.zeros((b * T, d), dtype=np.float32)
    for c in range(N_CORES):
        total += np.asarray(res.results[c]["out"], dtype=np.float32)

    out = np.zeros((b, n, d), dtype=np.float32)
    for i in range(b):
        out[i][idx[i]] = total[i * T: i * T + m[i]]
    return out


# revision 9
# speedup vs baseline: 1.3338x; 1.1117x over previous
"""Head-sharded tensor-parallel attention (2 heads/core, 8 cores).

Each core computes QKV for its 2 heads over ALL active tokens of both
batches, full attention for those heads, and a partial output
``AO_c @ W_out[c's 128 inner dims, :]``.  The host sums the 8 partial
outputs (the unshard step of the W_out-row sharding) and scatters back
into masked positions.  Masked-out rows of the reference output are
exactly zero, so only active tokens are processed (gathered on host);
pad keys have x=0 -> k=0 -> exp(0)=1, corrected by subtracting the pad
count from the softmax denominator.
"""

import math
from contextlib import ExitStack

import numpy as np
import ml_dtypes

import concourse.bass as bass
import concourse.mybir as mybir
import concourse.tile as tile
from concourse import bacc
from concourse.bass_utils import run_bass_kernel_spmd

P = 128
D = 1024          # model dim
HEADS = 16
DH = 64
VW = DH + 1       # v columns per head + ones column (softmax denominator)
SCALE = DH ** -0.5
N_CORES = 8
BF16 = mybir.dt.bfloat16
F32 = mybir.dt.float32


def _chunks(total, step):
    out = []
    o = 0
    while o < total:
        out.append((o, min(step, total - o)))
        o += step
    return out


def _build(T: int):
    """Per-core SPMD graph; T = padded token count per batch (mult of 128)."""
    nkt = T // P
    NT = 2 * T
    QCS = _chunks(T, 512)      # query chunks within a batch
    nmt = NT // P              # output token tiles

    nc = bacc.Bacc(None, target_bir_lowering=False, num_devices=N_CORES)

    xt_in = nc.declare_dram_parameter("xt", [D, NT], BF16, isOutput=False)
    wqkv_in = nc.declare_dram_parameter("wqkv", [D, 384], BF16, isOutput=False)
    wout_in = nc.declare_dram_parameter("wout", [P, D], BF16, isOutput=False)
    npad_in = nc.declare_dram_parameter("npad", [1, 2], F32, isOutput=False)
    out_ext = nc.declare_dram_parameter("out", [NT, D], BF16, isOutput=True)

    with tile.TileContext(nc) as tc, ExitStack() as ctx:
        sb = ctx.enter_context(tc.tile_pool(name="sb", bufs=1))
        ps = ctx.enter_context(tc.tile_pool(name="ps", bufs=1, space="PSUM"))

        npad_sb = sb.tile([1, 2], F32, tag="npad", bufs=1, name="npad_sb")
        nc.sync.dma_start(npad_sb[:], npad_in[:])

        # HAM warm-up: dependency-free matmuls so the PE clock ramps while
        # the first DMAs land.
        warm = sb.tile([P, 512], BF16, tag="warm", bufs=1, name="warm")
        nc.vector.memset(warm[:], 0.0)
        for i in range(18):
            wps = ps.tile([P, 512], F32, tag="ss", bufs=2, name=f"wps{i}")
            nc.tensor.matmul(wps[:], warm[:, 0:P], warm[:],
                             start=True, stop=True, skip_group_check=True)

        # ---- input DMAs, round-robined over sequencers.
        seqs = [nc.sync, nc.scalar, nc.gpsimd]
        _n = [0]

        def dma(dst, src, seq=None):
            (seqs[_n[0] % len(seqs)] if seq is None else seq).dma_start(dst, src)
            _n[0] += 1

        wqkv_sb = []
        for kc in range(8):
            tw = sb.tile([P, 384], BF16, tag="wqkv", bufs=8, name=f"wqkv{kc}")
            dma(tw[:], wqkv_in[kc * P:(kc + 1) * P, :])
            wqkv_sb.append(tw)
        xt = []
        for kc in range(8):
            t_ = sb.tile([P, NT], BF16, tag="xt", bufs=8, name=f"xt{kc}")
            xt.append(t_)
        # batch-0 columns land chunk-by-chunk so K/Q projections (and the
        # exp stream behind them) start before the full x^T arrives.
        for qo, qw in QCS:
            for kc in range(8):
                dma(xt[kc][:, qo:qo + qw], xt_in[kc * P:(kc + 1) * P, qo:qo + qw])
        for kc in range(8):
            dma(xt[kc][:, T:NT], xt_in[kc * P:(kc + 1) * P, T:NT])
        wout_sb = sb.tile([P, D], BF16, tag="wout", bufs=1, name="wout_sb")
        dma(wout_sb[:, 0:512], wout_in[:, 0:512])
        dma(wout_sb[:, 512:D], wout_in[:, 512:D])

        # ---- K^T and Q^T per batch: [128 dims(2 heads), T] bf16 in SBUF.
        kf = [None, None]
        qt = [None, None]

        def proj_kq(b):
            kfb = sb.tile([P, T], BF16, tag="kf", bufs=2, name=f"kf{b}")
            qtb = sb.tile([P, T], BF16, tag="qt", bufs=2, name=f"qt{b}")
            # chunk-major so S(kt<qc_end, qc0) unlocks after the first pair
            for qo, qw in QCS:
                for dst, col0 in ((kfb, 128), (qtb, 0)):
                    pps = ps.tile([P, 512], F32, tag="ss", bufs=2,
                                  name=f"pp{b}_{col0}_{qo}")
                    for i in range(8):
                        kc = (i + qo // 512) % 8
                        nc.tensor.matmul(
                            pps[:, 0:qw],
                            wqkv_sb[kc][:, col0:col0 + P],
                            xt[kc][:, b * T + qo: b * T + qo + qw],
                            start=(i == 0), stop=(i == 7))
                    nc.vector.tensor_copy(dst[:, qo:qo + qw], pps[:, 0:qw])
            kf[b] = kfb
            qt[b] = qtb

        # ---- V tiles per (batch, kt): [128 keys, 2*VW] bf16 with ones col.
        vt = [[None] * nkt, [None] * nkt]

        def proj_v(b):
            for kt in range(nkt):
                t_ = sb.tile([P, 2 * VW], BF16, tag="vt", bufs=2 * nkt,
                             name=f"vt{b}_{kt}")
                nc.gpsimd.memset(
                    t_[:].rearrange("p (h c) -> p h c", c=VW)[:, :, DH:DH + 1], 1.0)
                vps = ps.tile([P, P], F32, tag="ss", bufs=2, name=f"vps{b}_{kt}")
                for i in range(8):
                    kc = (i + kt) % 8
                    nc.tensor.matmul(
                        vps[:],
                        xt[kc][:, b * T + kt * P: b * T + (kt + 1) * P],
                        wqkv_sb[kc][:, 256:384],
                        start=(i == 0), stop=(i == 7))
                nc.vector.tensor_copy(
                    t_[:].rearrange("p (h c) -> p h c", c=VW)[:, :, 0:DH],
                    vps[:].rearrange("p (h c) -> p h c", c=DH))
                vt[b][kt] = t_

        # ---- S + exp stream for one batch.  S^T tiles [keys, queries] per
        # (kt, qchunk); both heads share one 2-bank psum tile; exp writes a
        # per-kt SBUF tile pt[b][kt] = [128, 2*T] bf16 (head-major halves).
        pt = [[None] * nkt, [None] * nkt]

        def s_exp(b):
            for kt in range(nkt):
                ptt = sb.tile([P, 2 * T], BF16, tag="pt", bufs=nkt + 5,
                              name=f"pt{b}_{kt}")
                pt[b][kt] = ptt
                for qo, qw in QCS:
                    sps = ps.tile([P, 1024], F32, tag="sps", bufs=2,
                                  name=f"sps{b}_{kt}_{qo}")
                    for h in range(2):
                        nc.tensor.matmul(
                            sps[:, h * 512: h * 512 + qw],
                            kf[b][h * DH:(h + 1) * DH, kt * P:(kt + 1) * P],
                            qt[b][h * DH:(h + 1) * DH, qo:qo + qw],
                            start=True, stop=True, skip_group_check=True)
                    nc.scalar.activation(
                        ptt[:].rearrange("p (u c) -> p u c", c=T)[:, 0:2, qo:qo + qw],
                        sps[:].rearrange("p (u c) -> p u c", c=512)[:, 0:2, 0:qw],
                        mybir.ActivationFunctionType.Exp, scale=SCALE)

        # ---- AV pass + normalize for one batch -> aoT[b] [128, T] bf16.
        aoT = [None, None]

        def av_norm(b):
            aob = sb.tile([P, T], BF16, tag="aoT", bufs=2, name=f"aoT{b}")
            aoT[b] = aob
            for h in range(2):
                av_ = sb.tile([VW, T], F32, tag="avs", bufs=2, name=f"avs{b}_{h}")
                tmpb = sb.tile([DH, T], BF16, tag="tmpb", bufs=2, name=f"tmpb{b}")
                for qo, qw in QCS:
                    avp = ps.tile([P, 512], F32, tag="av", bufs=2,
                                  name=f"avp{b}_{h}_{qo}")
                    for kt in range(nkt):
                        nc.tensor.matmul(
                            avp[0:VW, 0:qw],
                            vt[b][kt][:, h * VW:(h + 1) * VW],
                            pt[b][kt][:, h * T + qo: h * T + qo + qw],
                            start=(kt == 0), stop=(kt == nkt - 1),
                            skip_group_check=True)
                    nc.vector.tensor_copy(av_[:, qo:qo + qw], avp[0:VW, 0:qw])
                    # chunked normalize chain so out-proj tiles unlock early
                    den = sb.tile([1, 512], F32, tag="den", bufs=4,
                                  name=f"den{b}_{h}_{qo}")
                    nc.vector.tensor_scalar(den[:, 0:qw], av_[DH:DH + 1, qo:qo + qw],
                                            npad_sb[0:1, b:b + 1], None,
                                            op0=mybir.AluOpType.subtract)
                    rec = sb.tile([1, 512], F32, tag="rec", bufs=4,
                                  name=f"rec{b}_{h}_{qo}")
                    nc.vector.reciprocal_approx_fast(rec[:, 0:qw], den[:, 0:qw])
                    fac = sb.tile([DH, 512], F32, tag="fac", bufs=2,
                                  name=f"fac{b}_{h}_{qo}")
                    nc.gpsimd.partition_broadcast(fac[:, 0:qw], rec[:, 0:qw])
                    if h == 0:
                        nc.vector.tensor_tensor(aob[0:DH, qo:qo + qw],
                                                av_[0:DH, qo:qo + qw],
                                                fac[:, 0:qw],
                                                op=mybir.AluOpType.mult)
                    else:
                        nc.vector.tensor_tensor(tmpb[:, qo:qo + qw],
                                                av_[0:DH, qo:qo + qw],
                                                fac[:, 0:qw],
                                                op=mybir.AluOpType.mult)
                        # partition shift 0:64 -> 64:128 needs a DMA, not DVE
                        nc.sync.dma_start(aob[DH:P, qo:qo + qw],
                                          tmpb[:, qo:qo + qw])

        # ---- partial out-projection for one batch: [T, 1024] bf16 -> DRAM.
        def out_proj(b, use_scalar):
            for mt in range(nkt):
                osb = sb.tile([P, D], BF16, tag="osb", bufs=3, name=f"osb{b}_{mt}")
                for nf in range(2):
                    ops = ps.tile([P, 512], F32, tag="ss", bufs=2,
                                  name=f"op{b}_{mt}_{nf}")
                    nc.tensor.matmul(ops[:], aob_slice(b, mt),
                                     wout_sb[:, nf * 512:(nf + 1) * 512],
                                     start=True, stop=True, skip_group_check=True)
                    if nf == 1 and use_scalar:
                        # scalar engine is free of exp work by now
                        nc.scalar.activation(osb[:, 512:D], ops[:],
                                             mybir.ActivationFunctionType.Copy)
                    else:
                        nc.vector.tensor_copy(osb[:, nf * 512:(nf + 1) * 512],
                                              ops[:])
                dma(out_ext[b * T + mt * P: b * T + (mt + 1) * P, 0:512],
                    osb[:, 0:512], seq=nc.sync)
                dma(out_ext[b * T + mt * P: b * T + (mt + 1) * P, 512:D],
                    osb[:, 512:D], seq=nc.gpsimd)

        def aob_slice(b, mt):
            return aoT[b][:, mt * P:(mt + 1) * P]

        # ---- schedule (program order = scheduler priority)
        proj_kq(0)
        s_exp(0)       # streams on ACT while PE continues below
        proj_v(0)
        proj_kq(1)
        proj_v(1)
        s_exp(1)
        av_norm(0)     # runs under the exp(1) stream
        out_proj(0, use_scalar=False)   # fills the exp(1) tail
        av_norm(1)
        out_proj(1, use_scalar=True)

    nc.compile()
    return nc


_GRAPH_CACHE: dict = {}


def _get_graph(T: int):
    if T not in _GRAPH_CACHE:
        _GRAPH_CACHE[T] = _build(T)
    return _GRAPH_CACHE[T]


def kernel(x, mask, W_qkv, W_out):
    x = np.asarray(x, dtype=np.float32)
    mask = np.asarray(mask, dtype=np.float32)
    W_qkv = np.asarray(W_qkv, dtype=np.float32)
    W_out = np.asarray(W_out, dtype=np.float32)
    b, n, d = x.shape
    assert (b, d) == (2, D) and W_qkv.shape == (D, 3 * D)

    idx = [np.nonzero(mask[i] > 0.5)[0] for i in range(b)]
    m = [len(ix) for ix in idx]
    nkt = max(1, math.ceil(max(m) / P))
    T = nkt * P

    nc = _get_graph(T)

    bf16 = ml_dtypes.bfloat16
    xg = np.zeros((b, T, d), dtype=np.float32)
    for i in range(b):
        xg[i, :m[i]] = x[i][idx[i]]
    xt_all = np.ascontiguousarray(
        xg.reshape(b * T, d).transpose(1, 0)).astype(bf16)   # [D, 2T]
    npad = np.array([[T - m[0], T - m[1]]], dtype=np.float32)

    in_maps = []
    for c in range(N_CORES):
        cols = slice(c * P, (c + 1) * P)
        wqkv_c = np.ascontiguousarray(np.concatenate(
            [W_qkv[:, 0 * D:1 * D][:, cols],
             W_qkv[:, 1 * D:2 * D][:, cols],
             W_qkv[:, 2 * D:3 * D][:, cols]], axis=1)).astype(bf16)
        wout_c = np.ascontiguousarray(W_out[cols, :]).astype(bf16)
        in_maps.append({
            "xt": xt_all,
            "wqkv": wqkv_c,
            "wout": wout_c,
            "npad": npad,
        })

    res = run_bass_kernel_spmd(nc, in_maps, core_ids=list(range(N_CORES)))

    total = np.zeros((b * T, d), dtype=np.float32)
    for c in range(N_CORES):
        total += np.asarray(res.results[c]["out"], dtype=np.float32)

    out = np.zeros((b, n, d), dtype=np.float32)
    for i in range(b):
        out[i][idx[i]] = total[i * T: i * T + m[i]]
    return out
